# revision 54
# baseline (speedup 1.0000x reference)
"""Causal uniform attention (prefix-mean over sequence) for Trainium2.

out[b, s, :] = mean(x[b, 0:s+1, :])  for x of shape [8, 4096, 1024] f32.

Sharding: data-parallel over batch, one batch element per NeuronCore (8 cores).

Per-core algorithm (x_b [4096, 1024]):
  S is split into 33 blocks of 127 rows (last block 32 real rows), grouped as
  3 groups of 11 blocks. The host transposes each batch into a block-major
  HBM layout [128, 33*1024] (bf16): dram row p holds row p of every block
  back to back, so every group load / store is a plain 2-D slice with a
  22KB-contiguous run per partition (max DMA descriptor efficiency), and
  partition counts are always 128 (keeps the SDMA 16-way splitter engaged).
  SBUF partition 127 of each block slot holds that block's running prefix.

  Pipeline: ~3.5us of junk matmuls first (warms the PE's HAM clock gate to
  2.4 GHz), then phase-1 blocks of group g interleave with phase-3 blocks of
  group g-1 at a 2-block stagger, so the PE never idles long and the scaling
  engines are fed continuously from ~25us on.
    phase 1 (per group): 22 accumulating bf16 matmuls (ones-column lhsT
      patterns) -> PSUM [12, 1024]: row 0 = carry+group total, rows 1..11 =
      global exclusive block prefixes (a K=1 matmul folds in the previous
      group's carry). PSUM -> SBUF bf16 cast (alternating DVE/ACT), then one
      SBUF->SBUF HWDGE scatter (ACT ring) drops prefix j into partition 127
      of block j's slot.
    phase 3 (per block): matmul with lhsT [128, 127] = upper-triangular ones
      (within-block cumsum) + all-ones row 127 (broadcasts the prefix row)
      -> PSUM [127, 1024]; multiply by 1/(s+1) per partition while copying
      PSUM->SBUF bf16 (strict DVE/ACT alternation so the 3-deep PSUM pool
      ping-pongs); paired-block stores issued from SyncE (all input loads
      are issued first on SyncE, so the shared HW queue drains every input
      byte before any output byte - no interleaving stalls).

  All matmuls use bf16 inputs with f32 PSUM accumulation. I/O is bf16 in HBM
  (host converts); rel err ~3e-3 vs the 2e-2 budget.
"""

import sys

try:
    import concourse.bass  # noqa: F401
except ImportError:
    for _p in ("/root/.axon_site/_ro/trn_rl_repo", "/opt/trn_rl_repo"):
        if _p not in sys.path:
            sys.path.append(_p)

import numpy as np
import ml_dtypes

import concourse.bass as bass  # noqa: F401
import concourse.mybir as mybir
import concourse.tile as tile
from concourse import bacc
from concourse.bass_utils import run_bass_kernel_spmd

B, S, D = 8, 4096, 1024
RB = 127                  # data rows per block
NB = (S + RB - 1) // RB   # 33 blocks
GS = 11                   # blocks per group
NG = NB // GS             # 3 groups
H = 512                   # matmul free-dim half (PSUM bank limit for f32)
CW = (GS + 1) * (GS + 1)  # csum lhsT columns (incl. K=1 carry block)
F32 = mybir.dt.float32
BF16 = mybir.dt.bfloat16
NPBF16 = ml_dtypes.bfloat16


def _build_nc():
    nc = bacc.Bacc("TRN2", target_bir_lowering=False, debug=False, num_devices=8)
    # Block-major layout: dram[p, gi*D:(gi+1)*D] = row p of block gi.
    x = nc.dram_tensor("x", (128, NB * D), BF16, kind="ExternalInput")
    out = nc.dram_tensor("out", (128, NB * D), BF16, kind="ExternalOutput")

    with tile.TileContext(nc) as tc:
        with (
            tc.tile_pool(name="consts", bufs=1) as consts,
            tc.tile_pool(name="xg", bufs=NG) as xgp,
            tc.tile_pool(name="prefs", bufs=NG) as prefp,
            tc.tile_pool(name="og", bufs=NG) as ogp,
            tc.tile_pool(name="pp", bufs=1, space="PSUM") as ppool,
            tc.tile_pool(name="po", bufs=3, space="PSUM") as popool,
        ):
            # Constants are generated on-chip: DMAing 1-row-per-partition
            # layouts costs ~15us of tiny descriptors at kernel start.
            # utp: upper-triangular ones (within-block cumsum) + ones row 127.
            utp_f = consts.tile([128, RB], F32)
            nc.gpsimd.memset(utp_f[:], 1.0)
            nc.gpsimd.affine_select(
                out=utp_f[0:RB, :],
                in_=utp_f[0:RB, :],
                pattern=[[1, RB]],
                channel_multiplier=-1,
                base=0,
                compare_op=mybir.AluOpType.is_ge,
                fill=0.0,
            )
            sb_utp = consts.tile([128, RB], BF16)
            nc.vector.tensor_copy(sb_utp[:], utp_f[:])
            # csum cols [(GS+1)j, (GS+1)(j+1)): phase-1 lhsT for block j.
            # PSUM row 0 = carry+group total (every block contributes);
            # row 1+k = global excl prefix of block k (blocks j<k contribute).
            # Cols [(GS+1)GS, CW): all ones (K=1 carry-broadcast lhsT).
            csum_f = consts.tile([RB, CW], F32)
            nc.gpsimd.memset(csum_f[:], 0.0)
            for j in range(GS):
                c = (GS + 1) * j
                nc.gpsimd.memset(csum_f[:, c : c + 1], 1.0)
                if j + 2 <= GS:
                    nc.gpsimd.memset(csum_f[:, c + j + 2 : c + GS + 1], 1.0)
            nc.gpsimd.memset(csum_f[:, (GS + 1) * GS : CW], 1.0)
            sb_csum = consts.tile([RB, CW], BF16)
            nc.vector.tensor_copy(sb_csum[:], csum_f[:])
            # scales[p, i] = 1 / (127 i + p + 1)  (row 127 scales a pad row).
            sb_scint = consts.tile([128, NB], mybir.dt.int32)
            nc.gpsimd.iota(
                sb_scint[:], pattern=[[RB, NB]], base=1, channel_multiplier=1
            )
            sb_scf = consts.tile([128, NB], F32)
            nc.vector.tensor_copy(sb_scf[:], sb_scint[:])
            sb_scales = consts.tile([128, NB], F32)
            nc.vector.reciprocal(sb_scales[:], sb_scf[:])

            pref = []  # per-group [GS+1, 1024] tiles; row 0 = next carry
            xgs = []

            def load_in(g):
                # All input loads are emitted before any out-DMA issue so
                # SyncE's in-order stream (and the shared HW queue) never
                # parks an input behind an output's wait-for-scaling.
                xg = xgp.tile([128, GS * D], BF16, tag="xg")
                xgs.append(xg)
                # 3-block chunks: 6KB-contiguous descriptor runs keep DMA
                # near peak while giving phase 1 per-chunk completion
                # semaphores (a whole-group transfer would park the PE
                # stream behind 2.9MB; per-block loads issue too slowly).
                for c in range(0, GS, 3):
                    ce = min(c + 3, GS)
                    nc.sync.dma_start(
                        xg[:, c * D : ce * D],
                        x[:, (g * GS + c) * D : (g * GS + ce) * D],
                    )

            def phase1_block(g, j):
                xg, pp = xgs[g], pps[g]
                for h in range(2):
                    nc.tensor.matmul(
                        pp[:, h * H : (h + 1) * H],
                        lhsT=sb_csum[:, (GS + 1) * j : (GS + 1) * (j + 1)],
                        rhs=xg[0:RB, j * D + h * H : j * D + h * H + H],
                        start=(j == 0),
                        stop=(j == GS - 1 and g == 0),
                    )

            def phase1_finish(g):
                # Fold the previous group's carry in, cast the prefix rows to
                # bf16, scatter them into partition 127 of each block slot.
                pp = pps[g]
                if g > 0:
                    for h in range(2):
                        nc.tensor.matmul(
                            pp[:, h * H : (h + 1) * H],
                            lhsT=sb_csum[0:1, (GS + 1) * GS : CW],
                            rhs=pref[g - 1][0:1, h * H : (h + 1) * H],
                            start=False,
                            stop=True,
                        )
                pf = prefp.tile([GS + 1, D], BF16, tag="pf")
                # Alternate the PSUM->bf16 prefix cast between DVE and ACT so
                # it doesn't always queue behind the same engine's scalings.
                if g % 2 == 0:
                    nc.vector.tensor_copy(pf[:], pp[:])
                else:
                    nc.scalar.copy(pf[:], pp[:])
                pref.append(pf)
                # HWDGE scatter off ACT's ring: ~1.5us latency vs ~7us for the
                # gpsimd SWDGE path (Q7 descriptor emission is slow).
                nc.scalar.dma_start(xgs[g][127:128, :], pf[1 : GS + 1, :])

            def phase3_block(g, j):
                # Cumsum + prefix broadcast, scale, paired store.
                xg, og = xgs[g], ogs[g]
                gi = g * GS + j
                po = popool.tile([RB, D], F32, tag="po")
                for h in range(2):
                    nc.tensor.matmul(
                        po[:, h * H : (h + 1) * H],
                        lhsT=sb_utp[:],
                        rhs=xg[0:128, j * D + h * H : j * D + h * H + H],
                        start=True,
                        stop=True,
                    )
                sc = sb_scales[0:RB, gi : gi + 1]
                dst = og[0:RB, j * D : (j + 1) * D]
                # Strict per-block alternation: consecutive po tiles drain on
                # opposite engines, so the 3-deep po pool ping-pongs at
                # ~0.65us/block instead of serializing on one engine.
                if gi % 2 == 1:
                    nc.vector.tensor_scalar_mul(dst, po[:, :], sc)
                else:
                    nc.scalar.mul(dst, po[:, :], sc)
                # Paired stores (4KB descriptors); last block solo so the
                # tail store is small and starts immediately.
                if j % 2 == 1 or j == GS - 1:
                    j0 = j - 1 if j % 2 == 1 else j
                    nc.sync.dma_start(
                        out[:, (g * GS + j0) * D : (gi + 1) * D],
                        og[:, j0 * D : (j + 1) * D],
                    )

            pps = []
            ogs = []
            LAG = 2
            # PE warm-up: ~3.5us of junk matmuls on const data while the
            # first input chunks are still in flight. The HAM clock gate
            # needs ~3.4us of sustained activity before it grants 2.4 GHz;
            # without this, every phase-1 matmul of group 0 runs at half
            # clock and the whole backbone shifts right by ~5us.
            po_warm = popool.tile([RB, D], F32, tag="po")
            for _ in range(18):
                nc.tensor.matmul(
                    po_warm[0:1, 0:RB],
                    lhsT=sb_utp[:, 0:1],
                    rhs=sb_utp[0:128, :],
                    start=True,
                    stop=True,
                )
            for g in range(NG):
                load_in(g)
            # Block-interleaved emission with a LAG-block stagger: the PE
            # alternates phase-1 of group g (paced by the chunked input
            # stream) with phase-3 of group g-1 (data on-chip), keeping the
            # PE dense (HAM stays at full clock) and the scaling engines fed
            # continuously. The stagger gives the cast->scatter chain of
            # group g-1 time to land before its first phase-3 matmul.
            for g in range(NG + 1):
                if g < NG:
                    pp = ppool.tile([GS + 1, D], F32, tag="pp")
                    pps.append(pp)
                if g >= 1:
                    og = ogp.tile([128, GS * D], BF16, tag="og")
                    ogs.append(og)
                j0 = 0 if g < NG else LAG
                for j in range(j0, GS + LAG):
                    if g < NG and j < GS:
                        phase1_block(g, j)
                    if g >= 1 and j >= LAG:
                        phase3_block(g - 1, j - LAG)
                    if g < NG and j == GS - 1:
                        phase1_finish(g)

    nc.compile()
    return nc


_NC = None


def _prep_inputs(x):
    """FULL f32 x [B, S, D] -> per-core block-major bf16 in_maps."""
    xb = np.asarray(x).astype(NPBF16)
    xp = np.zeros((B, NB, 128, D), dtype=NPBF16)
    for i in range(NB):
        r0 = i * RB
        r1 = min(r0 + RB, S)
        xp[:, i, : r1 - r0] = xb[:, r0:r1]
    # [B, NB, 128, D] -> [B, 128, NB*D]  (block-major per partition)
    xp = np.ascontiguousarray(xp.transpose(0, 2, 1, 3)).reshape(B, 128, NB * D)
    return [{"x": xp[b]} for b in range(B)]


def _post(results):
    """Per-core block-major bf16 'out' buffers -> FULL f32 [B, S, D]."""
    outs = []
    for b in range(B):
        op = results[b]["out"].reshape(128, NB, D).transpose(1, 0, 2)
        outs.append(op[:, :RB].reshape(NB * RB, D)[:S])
    return np.stack(outs, axis=0).astype(np.float32)


def kernel(x):
    global _NC
    assert x.shape == (B, S, D)
    if _NC is None:
        _NC = _build_nc()
    res = run_bass_kernel_spmd(_NC, _prep_inputs(x), core_ids=list(range(B)))
    return _post(res.results)


# revision 55
# speedup vs baseline: 1.1939x; 1.1939x over previous
"""Causal uniform attention (prefix-mean over sequence) for Trainium2.

out[b, s, :] = mean(x[b, 0:s+1, :])  for x of shape [8, 4096, 1024] f32.

Sharding: data-parallel over batch, one batch element per NeuronCore (8 cores).

Per-core algorithm (x_b [4096, 1024]):
  S is split into 33 blocks of 127 rows (last block 32 real rows), grouped as
  3 groups of 11 blocks. The host transposes each batch into a block-major
  HBM layout [128, 33*1024] (bf16): dram row p holds row p of every block
  back to back, so every group load / store is a plain 2-D slice with a
  22KB-contiguous run per partition (max DMA descriptor efficiency), and
  partition counts are always 128 (keeps the SDMA 16-way splitter engaged).
  SBUF partition 127 of each block slot holds that block's running prefix.

  Pipeline: ~3.5us of junk matmuls first (warms the PE's HAM clock gate to
  2.4 GHz), then phase-1 blocks of group g interleave with phase-3 blocks of
  group g-1 at a 2-block stagger, so the PE never idles long and the scaling
  engines are fed continuously from ~25us on.
    phase 1 (per group): 22 accumulating bf16 matmuls (ones-column lhsT
      patterns) -> PSUM [12, 1024]: row 0 = carry+group total, rows 1..11 =
      global exclusive block prefixes (a K=1 matmul folds in the previous
      group's carry). PSUM -> SBUF bf16 cast (alternating DVE/ACT), then one
      SBUF->SBUF HWDGE scatter (ACT ring) drops prefix j into partition 127
      of block j's slot.
    phase 3 (per block): matmul with lhsT [128, 127] = upper-triangular ones
      (within-block cumsum) + all-ones row 127 (broadcasts the prefix row)
      -> PSUM [127, 1024]; multiply by 1/(s+1) per partition while copying
      PSUM->SBUF bf16 (strict DVE/ACT alternation so the 3-deep PSUM pool
      ping-pongs); paired-block stores issued from SyncE (all input loads
      are issued first on SyncE, so the shared HW queue drains every input
      byte before any output byte - no interleaving stalls).

  All matmuls use bf16 inputs with f32 PSUM accumulation. I/O is bf16 in HBM
  (host converts); rel err ~3e-3 vs the 2e-2 budget.
"""

import sys

try:
    import concourse.bass  # noqa: F401
except ImportError:
    for _p in ("/root/.axon_site/_ro/trn_rl_repo", "/opt/trn_rl_repo"):
        if _p not in sys.path:
            sys.path.append(_p)

import numpy as np
import ml_dtypes

import concourse.bass as bass  # noqa: F401
import concourse.mybir as mybir
import concourse.tile as tile
from concourse import bacc
from concourse.bass_utils import run_bass_kernel_spmd

B, S, D = 8, 4096, 1024
RB = 127                  # data rows per block
NB = (S + RB - 1) // RB   # 33 blocks
GS = 11                   # blocks per group
NG = NB // GS             # 3 groups
H = 512                   # matmul free-dim half (PSUM bank limit for f32)
CW = (GS + 1) * (GS + 1)  # csum lhsT columns (incl. K=1 carry block)
F32 = mybir.dt.float32
BF16 = mybir.dt.bfloat16
NPBF16 = ml_dtypes.bfloat16


def _build_nc():
    nc = bacc.Bacc("TRN2", target_bir_lowering=False, debug=False, num_devices=8)
    # Block-major layout: dram[p, gi*D:(gi+1)*D] = row p of block gi.
    x = nc.dram_tensor("x", (128, NB * D), BF16, kind="ExternalInput")
    out = nc.dram_tensor("out", (128, NB * D), BF16, kind="ExternalOutput")

    with tile.TileContext(nc) as tc:
        with (
            tc.tile_pool(name="consts", bufs=1) as consts,
            tc.tile_pool(name="xg", bufs=NG) as xgp,
            tc.tile_pool(name="prefs", bufs=NG) as prefp,
            tc.tile_pool(name="og", bufs=NG) as ogp,
            tc.tile_pool(name="pp", bufs=1, space="PSUM") as ppool,
            tc.tile_pool(name="po", bufs=3, space="PSUM") as popool,
        ):
            # Constants are generated on-chip: DMAing 1-row-per-partition
            # layouts costs ~15us of tiny descriptors at kernel start.
            # utp: upper-triangular ones (within-block cumsum) + ones row 127.
            utp_f = consts.tile([128, RB], F32)
            nc.gpsimd.memset(utp_f[:], 1.0)
            nc.gpsimd.affine_select(
                out=utp_f[0:RB, :],
                in_=utp_f[0:RB, :],
                pattern=[[1, RB]],
                channel_multiplier=-1,
                base=0,
                compare_op=mybir.AluOpType.is_ge,
                fill=0.0,
            )
            sb_utp = consts.tile([128, RB], BF16)
            nc.vector.tensor_copy(sb_utp[:], utp_f[:])
            # csum cols [(GS+1)j, (GS+1)(j+1)): phase-1 lhsT for block j.
            # PSUM row 0 = carry+group total (every block contributes);
            # row 1+k = global excl prefix of block k (blocks j<k contribute).
            # Cols [(GS+1)GS, CW): all ones (K=1 carry-broadcast lhsT).
            csum_f = consts.tile([RB, CW], F32)
            nc.gpsimd.memset(csum_f[:], 0.0)
            for j in range(GS):
                c = (GS + 1) * j
                nc.gpsimd.memset(csum_f[:, c : c + 1], 1.0)
                if j + 2 <= GS:
                    nc.gpsimd.memset(csum_f[:, c + j + 2 : c + GS + 1], 1.0)
            nc.gpsimd.memset(csum_f[:, (GS + 1) * GS : CW], 1.0)
            sb_csum = consts.tile([RB, CW], BF16)
            nc.vector.tensor_copy(sb_csum[:], csum_f[:])
            # scales[p, i] = 1 / (127 i + p + 1)  (row 127 scales a pad row).
            sb_scint = consts.tile([128, NB], mybir.dt.int32)
            nc.gpsimd.iota(
                sb_scint[:], pattern=[[RB, NB]], base=1, channel_multiplier=1
            )
            sb_scf = consts.tile([128, NB], F32)
            nc.vector.tensor_copy(sb_scf[:], sb_scint[:])
            sb_scales = consts.tile([128, NB], F32)
            nc.vector.reciprocal(sb_scales[:], sb_scf[:])

            pref = []  # per-group [GS+1, 1024] tiles; row 0 = next carry
            xgs = []

            def load_in(g):
                # All input loads are emitted before any out-DMA issue so
                # SyncE's in-order stream (and the shared HW queue) never
                # parks an input behind an output's wait-for-scaling.
                xg = xgp.tile([128, GS * D], BF16, tag="xg")
                xgs.append(xg)
                # 3-block chunks: 6KB-contiguous descriptor runs keep DMA
                # near peak while giving phase 1 per-chunk completion
                # semaphores (a whole-group transfer would park the PE
                # stream behind 2.9MB; per-block loads issue too slowly).
                for c in range(0, GS, 3):
                    ce = min(c + 3, GS)
                    nc.sync.dma_start(
                        xg[:, c * D : ce * D],
                        x[:, (g * GS + c) * D : (g * GS + ce) * D],
                    )

            def phase1_block(g, j):
                xg, pp = xgs[g], pps[g]
                for h in range(2):
                    nc.tensor.matmul(
                        pp[:, h * H : (h + 1) * H],
                        lhsT=sb_csum[:, (GS + 1) * j : (GS + 1) * (j + 1)],
                        rhs=xg[0:RB, j * D + h * H : j * D + h * H + H],
                        start=(j == 0),
                        stop=(j == GS - 1 and g == 0),
                    )

            def phase1_finish(g):
                # Fold the previous group's carry in, cast the prefix rows to
                # bf16, scatter them into partition 127 of each block slot.
                pp = pps[g]
                if g > 0:
                    for h in range(2):
                        nc.tensor.matmul(
                            pp[:, h * H : (h + 1) * H],
                            lhsT=sb_csum[0:1, (GS + 1) * GS : CW],
                            rhs=pref[g - 1][0:1, h * H : (h + 1) * H],
                            start=False,
                            stop=True,
                        )
                pf = prefp.tile([GS + 1, D], BF16, tag="pf")
                # Alternate the PSUM->bf16 prefix cast between DVE and ACT so
                # it doesn't always queue behind the same engine's scalings.
                if g % 2 == 0:
                    nc.vector.tensor_copy(pf[:], pp[:])
                else:
                    nc.scalar.copy(pf[:], pp[:])
                pref.append(pf)
                # HWDGE scatter off ACT's ring: ~1.5us latency vs ~7us for the
                # gpsimd SWDGE path (Q7 descriptor emission is slow).
                nc.scalar.dma_start(xgs[g][127:128, :], pf[1 : GS + 1, :])

            def phase3_block(g, j):
                # Cumsum + prefix broadcast, scale, paired store.
                xg, og = xgs[g], ogs[g]
                gi = g * GS + j
                po = popool.tile([RB, D], F32, tag="po")
                for h in range(2):
                    nc.tensor.matmul(
                        po[:, h * H : (h + 1) * H],
                        lhsT=sb_utp[:],
                        rhs=xg[0:128, j * D + h * H : j * D + h * H + H],
                        start=True,
                        stop=True,
                    )
                sc = sb_scales[0:RB, gi : gi + 1]
                dst = og[0:RB, j * D : (j + 1) * D]
                # Strict per-block alternation: consecutive po tiles drain on
                # opposite engines, so the 3-deep po pool ping-pongs at
                # ~0.65us/block instead of serializing on one engine.
                if gi % 2 == 1:
                    nc.vector.tensor_scalar_mul(dst, po[:, :], sc)
                else:
                    nc.scalar.mul(dst, po[:, :], sc)
                # Paired stores (4KB descriptors); last block solo so the
                # tail store is small and starts immediately.
                if j % 2 == 1 or j == GS - 1:
                    j0 = j - 1 if j % 2 == 1 else j
                    nc.sync.dma_start(
                        out[:, (g * GS + j0) * D : (gi + 1) * D],
                        og[:, j0 * D : (j + 1) * D],
                    )

            pps = []
            ogs = []
            LAG = 2
            # PE warm-up: ~3.5us of junk matmuls on const data while the
            # first input chunks are still in flight. The HAM clock gate
            # needs ~3.4us of sustained activity before it grants 2.4 GHz;
            # without this, every phase-1 matmul of group 0 runs at half
            # clock and the whole backbone shifts right by ~5us.
            po_warm = popool.tile([RB, D], F32, tag="po")
            for _ in range(30):
                nc.tensor.matmul(
                    po_warm[0:1, 0:RB],
                    lhsT=sb_utp[:, 0:1],
                    rhs=sb_utp[0:128, :],
                    start=True,
                    stop=True,
                )
            for g in range(NG):
                load_in(g)
            # Block-interleaved emission with a LAG-block stagger: the PE
            # alternates phase-1 of group g (paced by the chunked input
            # stream) with phase-3 of group g-1 (data on-chip), keeping the
            # PE dense (HAM stays at full clock) and the scaling engines fed
            # continuously. The stagger gives the cast->scatter chain of
            # group g-1 time to land before its first phase-3 matmul.
            for g in range(NG + 1):
                if g < NG:
                    pp = ppool.tile([GS + 1, D], F32, tag="pp")
                    pps.append(pp)
                if g >= 1:
                    og = ogp.tile([128, GS * D], BF16, tag="og")
                    ogs.append(og)
                j0 = 0 if g < NG else LAG
                for j in range(j0, GS + LAG):
                    if g < NG and j < GS:
                        phase1_block(g, j)
                    if g >= 1 and j >= LAG:
                        phase3_block(g - 1, j - LAG)
                    if g < NG and j == GS - 1:
                        phase1_finish(g)

    nc.compile()
    return nc


_NC = None


def _prep_inputs(x):
    """FULL f32 x [B, S, D] -> per-core block-major bf16 in_maps."""
    xb = np.asarray(x).astype(NPBF16)
    xp = np.zeros((B, NB, 128, D), dtype=NPBF16)
    for i in range(NB):
        r0 = i * RB
        r1 = min(r0 + RB, S)
        xp[:, i, : r1 - r0] = xb[:, r0:r1]
    # [B, NB, 128, D] -> [B, 128, NB*D]  (block-major per partition)
    xp = np.ascontiguousarray(xp.transpose(0, 2, 1, 3)).reshape(B, 128, NB * D)
    return [{"x": xp[b]} for b in range(B)]


def _post(results):
    """Per-core block-major bf16 'out' buffers -> FULL f32 [B, S, D]."""
    outs = []
    for b in range(B):
        op = results[b]["out"].reshape(128, NB, D).transpose(1, 0, 2)
        outs.append(op[:, :RB].reshape(NB * RB, D)[:S])
    return np.stack(outs, axis=0).astype(np.float32)


def kernel(x):
    global _NC
    assert x.shape == (B, S, D)
    if _NC is None:
        _NC = _build_nc()
    res = run_bass_kernel_spmd(_NC, _prep_inputs(x), core_ids=list(range(B)))
    return _post(res.results)
